# revision 6
# baseline (speedup 1.0000x reference)
"""Multi-head attention (dense transformer block) on 8 Trainium2 NeuronCores.

Sharding: one head per core; per-core partial output projections summed on
host (no device-to-device comm). b_o enters via woa row 0 (core 0 only).

v2 pipeline (vs v1 baseline):
  - scores matmul (K=hd=64) row-tiled 2-up: even jb blocks in PE rows 0-63,
    odd jb in rows 64-127 (kTd/qTd carry duplicated/interleaved partition
    halves), halving scores PE time.
  - q,k projection merged into one M=128 matmul (psum rows 0-63 = q,
    64-127 = k) with evictions landing directly in the packed layouts.
  - QKV chunks for later data interleaved into earlier attention windows
    (fills PE idle slots of the ACT-bound phase).
  - softmax l: ones column at M index 0 of vaug; normalization applied to
    Ou^T before the output projection via gpsimd partition_broadcast of
    1/l and a DVE broadcast-multiply (kills v1's 64 tiny transpose DMAs).
  - batched DMAs: 4-per-b xt loads, 1-per-window output store (fp16).
"""
import numpy as np
from contextlib import ExitStack

import concourse.bass as bass
import concourse.tile as tile
from concourse import bacc, mybir
from concourse.bass_utils import run_bass_kernel_spmd
from concourse.masks import make_identity

dt = mybir.dt

H = 8
HD = 64
D = 512
B = 2
N = 4096
NB = B * N
NJB = N // 128      # 32 key blocks per batch
NPAIR = NJB // 2    # 16 jb pairs
NCH = N // 512      # 8 qkv chunks per batch
W = 1024            # i-window (exp width, 2 PSUM banks)
NW = N // W         # 4 windows per batch
SCALE = 0.125
SHIFT = 2.0  # global logit shift; exact for softmax, keeps exp() in fp16 range

MM_DT = dt.float16
MM_NP = np.float16


def _build(repeat=1, pvlag=2):
    nc = bacc.Bacc("TRN2", target_bir_lowering=False, debug=False, num_devices=8)
    xt = nc.dram_tensor("xt", [D, NB], MM_DT, kind="ExternalInput").ap()
    wqkvt = nc.dram_tensor("wqkvt", [D, 3 * HD], MM_DT, kind="ExternalInput").ap()
    woat = nc.dram_tensor("woat", [HD + 1, D], MM_DT, kind="ExternalInput").ap()
    part = nc.dram_tensor("part", [B, N, D], MM_DT, kind="ExternalOutput").ap()

    with tile.TileContext(nc) as tc:
        with ExitStack() as ctx:
            const_p = ctx.enter_context(tc.tile_pool(name="const", bufs=1))
            xt_p = ctx.enter_context(tc.tile_pool(name="xt", bufs=1))
            qkv_p = ctx.enter_context(tc.tile_pool(name="qkv", bufs=1))
            vaug_p = ctx.enter_context(tc.tile_pool(name="vaug", bufs=1))
            pt_p = ctx.enter_context(tc.tile_pool(name="pt", bufs=8))
            ot_p = ctx.enter_context(tc.tile_pool(name="ot", bufs=4))
            lr_p = ctx.enter_context(tc.tile_pool(name="lr", bufs=4))
            osb_p = ctx.enter_context(tc.tile_pool(name="osb", bufs=2))
            spool = ctx.enter_context(tc.tile_pool(name="spool", bufs=2, space="PSUM"))
            vpool = ctx.enter_context(tc.tile_pool(name="vpool", bufs=2, space="PSUM"))
            projpool = ctx.enter_context(tc.tile_pool(name="projpool", bufs=2, space="PSUM"))

            ident = const_p.tile([128, 128], MM_DT, tag="ident")
            make_identity(nc, ident[:])
            wq = const_p.tile([128, 4, 3 * HD], MM_DT, tag="wq")
            for d in range(4):
                nc.sync.dma_start(wq[:, d, :], wqkvt[d * 128:(d + 1) * 128, :])
            woa = const_p.tile([HD + 1, D], MM_DT, tag="woa")
            nc.sync.dma_start(woa[:], woat[:])
            shiftc = const_p.tile([128, 1], dt.float32, tag="shiftc")
            nc.vector.memset(shiftc[:], -SHIFT)

            # packed layouts for row-tiled scores
            qTd = qkv_p.tile([128, NB], MM_DT, tag="qTd")     # 0:64 qT, 64:128 dup
            kTd = qkv_p.tile([128, NB // 2], MM_DT, tag="kTd")  # 0:64 even jb, 64:128 odd
            vT = qkv_p.tile([64, NB], MM_DT, tag="vT")
            ktmp = qkv_p.tile([128, N // 2], MM_DT, tag="ktmp")  # rows 64:128 staging
            xts = [xt_p.tile([128, 4, N], MM_DT, tag=f"xt{b}", name=f"xt{b}")
                   for b in range(B)]
            vaug = [vaug_p.tile([128, NJB * 65], MM_DT, tag=f"vaug{b}", name=f"vaug{b}")
                    for b in range(B)]

            def emit_xt_loads(b):
                for q in range(4):
                    nc.sync.dma_start(
                        xts[b][:, :, q * 1024:(q + 1) * 1024],
                        xt[:, b * N + q * 1024:b * N + (q + 1) * 1024]
                        .rearrange("(d p) t -> p d t", p=128))

            def emit_chunk(b, ch):
                """QKV projection + layout evictions + vaug for jb 4ch..4ch+3."""
                c0 = ch * 512
                ps_qk = spool.tile([128, W], dt.float32, tag="s", name="ps_qk")
                for d in range(4):
                    nc.tensor.matmul(ps_qk[:, 0:512], wq[:, d, 0:128],
                                     xts[b][:, d, c0:c0 + 512],
                                     start=(d == 0), stop=(d == 3))
                ps_v = projpool.tile([128, 512], dt.float32, tag="pj", name="ps_v")
                for d in range(4):
                    nc.tensor.matmul(ps_v[0:64, :], wq[:, d, 128:192],
                                     xts[b][:, d, c0:c0 + 512],
                                     start=(d == 0), stop=(d == 3))
                nc.vector.tensor_copy(qTd[0:64, b * N + c0:b * N + c0 + 512],
                                      ps_qk[0:64, 0:512])
                for t in range(2):
                    pr = 2 * ch + t
                    nc.vector.tensor_copy(
                        kTd[64:128, b * (N // 2) + pr * 128:b * (N // 2) + (pr + 1) * 128],
                        ps_qk[64:128, (2 * t + 1) * 128:(2 * t + 2) * 128])
                    nc.vector.tensor_copy(
                        ktmp[64:128, pr * 128:(pr + 1) * 128],
                        ps_qk[64:128, (2 * t) * 128:(2 * t + 1) * 128])
                nc.vector.tensor_copy(vT[0:64, b * N + c0:b * N + c0 + 512],
                                      ps_v[0:64, :])
                nc.sync.dma_start(
                    kTd[0:64, b * (N // 2) + 2 * ch * 128:b * (N // 2) + 2 * ch * 128 + 256],
                    ktmp[64:128, 2 * ch * 128:2 * ch * 128 + 256])
                nc.sync.dma_start(qTd[64:128, b * N + c0:b * N + c0 + 512],
                                  qTd[0:64, b * N + c0:b * N + c0 + 512])
                for j in range(4):
                    jb = 4 * ch + j
                    tr = projpool.tile([128, 64], MM_DT, tag="pj", name="tr")
                    nc.tensor.transpose(tr[:], vT[:, b * N + jb * 128:b * N + (jb + 1) * 128],
                                        ident[0:64, 0:64])
                    nc.vector.tensor_copy(vaug[b][:, jb * 65 + 1:jb * 65 + 65], tr[:])

            def emit_divproj(b, w, chains):
                osb_t = osb_p.tile([128, 8, 512], MM_DT, tag="osb", name="osb")
                for m, ps_c in enumerate(chains):
                    lrec = lr_p.tile([1, 512], dt.float32, tag="lrec", name="lrec")
                    nc.vector.reciprocal(lrec[0:1, :], ps_c[0:1, :])
                    lrecb = lr_p.tile([HD + 1, 512], dt.float32, tag="lrecb", name="lrecb")
                    nc.gpsimd.partition_broadcast(lrecb[:], lrec[0:1, :])
                    ouTm = ot_p.tile([HD + 1, 512], MM_DT, tag="ot", name="ouT")
                    nc.vector.tensor_tensor(ouTm[:], ps_c[:], lrecb[:],
                                            mybir.AluOpType.mult)
                    for ib in range(4):
                        ps_p = projpool.tile([128, 512], dt.float32, tag="pj", name="ps_p")
                        nc.tensor.matmul(ps_p[:], ouTm[:, ib * 128:(ib + 1) * 128],
                                         woa[:], start=True, stop=True)
                        nc.vector.tensor_copy(osb_t[:, m * 4 + ib, :], ps_p[:])
                nc.sync.dma_start(
                    part[b, w * W:(w + 1) * W, :].rearrange("(ib p) e -> p ib e", p=128),
                    osb_t[:])

            def body(_=None):
                for b in range(B):
                    nc.vector.memset(vaug[b][:], 1.0)
                emit_xt_loads(0)
                emit_xt_loads(1)
                emit_chunk(0, 0)
                emit_chunk(0, 1)
                chunk_queue = [(0, ch) for ch in range(2, NCH)]
                chunk_queue += [(1, ch) for ch in range(NCH)]
                emitted_b0 = 2
                pending = []  # div/proj closures

                for b in range(B):
                    for w in range(NW):
                        ps_c0 = vpool.tile([HD + 1, 512], dt.float32, tag="pv", name="c0")
                        ps_c1 = vpool.tile([HD + 1, 512], dt.float32, tag="pv", name="c1")
                        pts = {}
                        for p in range(NPAIR):
                            ps_e = spool.tile([128, W], dt.float32, tag="s", name="ps_e")
                            ps_o = spool.tile([128, W], dt.float32, tag="s", name="ps_o")
                            kcol = b * (N // 2) + p * 128
                            for m in range(2):
                                qcol = b * N + w * W + m * 512
                                nc.tensor.matmul(ps_e[:, m * 512:(m + 1) * 512],
                                                 kTd[0:64, kcol:kcol + 128],
                                                 qTd[0:64, qcol:qcol + 512],
                                                 start=True, stop=True)
                                nc.tensor.matmul(ps_o[:, m * 512:(m + 1) * 512],
                                                 kTd[64:128, kcol:kcol + 128],
                                                 qTd[64:128, qcol:qcol + 512],
                                                 start=True, stop=True)
                            pt_e = pt_p.tile([128, W], MM_DT, tag="pt", name="pt_e")
                            pt_o = pt_p.tile([128, W], MM_DT, tag="pt", name="pt_o")
                            nc.scalar.activation(pt_e[:], ps_e[:],
                                                 mybir.ActivationFunctionType.Exp,
                                                 bias=shiftc[:, 0:1], scale=SCALE)
                            nc.scalar.activation(pt_o[:], ps_o[:],
                                                 mybir.ActivationFunctionType.Exp,
                                                 bias=shiftc[:, 0:1], scale=SCALE)
                            pts[2 * p] = pt_e
                            pts[2 * p + 1] = pt_o

                            # paced QKV chunk interleave
                            if p % 2 == 0 and chunk_queue:
                                need = (b == 0 and w == 0)
                                if need and emitted_b0 <= min(7, p // 2 + 1):
                                    cb, cc = chunk_queue.pop(0)
                                    emit_chunk(cb, cc)
                                    emitted_b0 += 1
                                elif not need:
                                    cb, cc = chunk_queue.pop(0)
                                    emit_chunk(cb, cc)
                                    if cb == 0:
                                        emitted_b0 += 1
                            # PV with lag
                            if p >= pvlag:
                                for jb in (2 * (p - pvlag), 2 * (p - pvlag) + 1):
                                    nc.tensor.matmul(ps_c0[:], vaug[b][:, jb * 65:(jb + 1) * 65],
                                                     pts[jb][:, 0:512],
                                                     start=(jb == 0), stop=(jb == NJB - 1))
                                    nc.tensor.matmul(ps_c1[:], vaug[b][:, jb * 65:(jb + 1) * 65],
                                                     pts[jb][:, 512:1024],
                                                     start=(jb == 0), stop=(jb == NJB - 1))
                            if p == 2 and pending:
                                pending.pop(0)()
                        for p in range(NPAIR - pvlag, NPAIR):
                            for jb in (2 * p, 2 * p + 1):
                                nc.tensor.matmul(ps_c0[:], vaug[b][:, jb * 65:(jb + 1) * 65],
                                                 pts[jb][:, 0:512],
                                                 start=(jb == 0), stop=(jb == NJB - 1))
                                nc.tensor.matmul(ps_c1[:], vaug[b][:, jb * 65:(jb + 1) * 65],
                                                 pts[jb][:, 512:1024],
                                                 start=(jb == 0), stop=(jb == NJB - 1))
                        pending.append(
                            lambda b=b, w=w, c=(ps_c0, ps_c1): emit_divproj(b, w, c))
                while pending:
                    pending.pop(0)()

            if repeat == 1:
                body()
            else:
                with tc.For_i(0, repeat, 1) as _i:
                    body()

    nc.compile()
    return nc


def _make_in_maps(x, w_qkv, w_o, b_o):
    xt = np.ascontiguousarray(x.transpose(2, 1, 0).reshape(D, B * N)).astype(MM_NP)
    in_maps = []
    for c in range(8):
        wqs = w_qkv[c * HD:(c + 1) * HD]
        wks = w_qkv[D + c * HD:D + (c + 1) * HD]
        wvs = w_qkv[2 * D + c * HD:2 * D + (c + 1) * HD]
        wqkvt = np.ascontiguousarray(np.concatenate([wqs, wks, wvs], 0).T).astype(MM_NP)
        bo_row = b_o if c == 0 else np.zeros_like(b_o)
        woat = np.concatenate(
            [bo_row[None, :], w_o[:, c * HD:(c + 1) * HD].T], 0).astype(MM_NP)
        in_maps.append({"xt": xt, "wqkvt": wqkvt, "woat": woat})
    return in_maps


_NC_CACHE = {}


def _get_nc(repeat=1, **kw):
    key = (repeat, tuple(sorted(kw.items())))
    if key not in _NC_CACHE:
        _NC_CACHE[key] = _build(repeat=repeat, **kw)
    return _NC_CACHE[key]


def kernel(x, w_qkv, w_o, b_o):
    x = np.asarray(x, np.float32)
    w_qkv = np.asarray(w_qkv, np.float32)
    w_o = np.asarray(w_o, np.float32)
    b_o = np.asarray(b_o, np.float32)
    assert x.shape == (N, B, D), x.shape
    nc = _get_nc()
    in_maps = _make_in_maps(x, w_qkv, w_o, b_o)
    res = run_bass_kernel_spmd(nc, in_maps, list(range(8)))
    acc = np.zeros((B, N, D), np.float64)
    for r in res.results:
        acc += r["part"].astype(np.float64)
    return acc.astype(np.float32)
